# revision 29
# baseline (speedup 1.0000x reference)
"""KPConv (nn_KPConvFPN) Trainium2 Bass kernel — v2.

Sharding: 8 cores; core c handles batch b=c//2, query half (c%2)*8192.
Host prep packs a combined gather table in DRAM (input): row m (256B) =
[64 x fp16 feat | sx,sy,sz as fp32 at f32-cols 32..34 | pad], plus exact
per-query neighbor counts (computed host-side from inputs).

Per-core pipeline, per kw-group kg (1024 queries, 16 gather chunks):
  1. one idx DMA [128, 1024] i16; 16 x dma_gather (SWDGE) of 1024 rows
     each, round-robin over 4 queues (queues run on distinct Q7 core
     pairs and overlap ~3x).
  2. rel3 [128, 3, GG] = s - q (SoA so ACT reads are contiguous).
  3. kw[n,k,p] = relu(1 - sqrt(|rel - kp_p|^2)/sigma) via ACT/DVE,
     fp16 output.
  4. kbd = kw * blockdiag-mask (fp16), einsum1 on PE per 8-query block:
     lhsT = gathered feats fp16, rhs = kbd -> PSUM, evict fp16 to wtt.
  5. einsum2 on PE fp16: 15 x (W_p stationary) + bias x cnt row,
     accumulate [o=128, q=512] in PSUM.
  6. PE-transpose 128-col chunks; ACT copy with per-partition scale
     1/cnt; store.
"""
import json

import numpy as np

import concourse.bass as bass
import concourse.mybir as mybir
from concourse.tile import TileContext
from concourse import library_config
from concourse import bass2jax

F32 = mybir.dt.float32
F16 = mybir.dt.float16
I16 = mybir.dt.int16

B, N, M, K = 4, 16384, 16384, 16
C_IN, C_OUT, P = 64, 128, 15
SIGMA = 0.03
N_CORES = 8
NQ_CORE = N // 2            # 8192 queries per core
NK_CORE = NQ_CORE * K       # 131072 gathered rows per core
ST_Q = 512                  # queries per supertile
N_ST = NQ_CORE // ST_Q      # 16
KW_ST = 2                   # supertiles per kw group
N_KG = N_ST // KW_ST        # 8
GG = KW_ST * ST_Q * K // 128  # 128 g-cols per kw group
G_ST = GG // KW_ST          # 64 g-cols per supertile
ROW16 = 128                 # fp16 units per table row (256B)

# ---------------------------------------------------------------------------
# walrus workaround: this nix walrus build supports ONE sync-wait per
# instruction; split extra waits onto NoOps inserted before the offender
# (same-engine program order preserves semantics). Also run
# codegen_inst_isa_subclasses (Bacc does; raw Bass doesn't) so extended
# instructions get their ISA bytes.
_orig_to_json_bytes = bass.Bass.to_json_bytes


def _fix_block(bb, ctr):
    insts = bb.get("instructions")
    if not isinstance(insts, list):
        return
    new = []
    for inst in insts:
        si = inst.get("sync_info")
        ow = si.get("on_wait") if isinstance(si, dict) else None
        if ow and len(ow) > 1:
            for w in ow[:-1]:
                ctr[0] += 1
                nop = {"engine": inst["engine"], "ins": [], "outs": [],
                       "name": f"I-wsplit-{ctr[0]}", "opcode": "NoOp",
                       "sync_info": {"on_update": [], "on_wait": [w]},
                       "text_hint": "wsplit"}
                if "debug" in inst:
                    nop["debug"] = inst["debug"]
                new.append(nop)
            si["on_wait"] = [ow[-1]]
        new.append(inst)
    bb["instructions"] = new


def _walk(o, ctr):
    if isinstance(o, dict):
        if isinstance(o.get("instructions"), list):
            _fix_block(o, ctr)
        for v in o.values():
            _walk(v, ctr)
    elif isinstance(o, list):
        for v in o:
            _walk(v, ctr)


def _to_json_bytes_split(self):
    mybir.codegen_inst_isa_subclasses(self)
    raw = _orig_to_json_bytes(self)
    d = json.loads(raw)
    ctr = [0]
    _walk(d, ctr)
    return json.dumps(d).encode()


bass.Bass.to_json_bytes = _to_json_bytes_split


def ap_view(t_ap, extra_offset, dims):
    """AP over tile t_ap with explicit free dims [[step, count], ...]
    (steps in elements); partition dim is taken from the tile."""
    return bass.AP(t_ap.tensor, t_ap.offset + extra_offset,
                   [t_ap.ap[0]] + list(dims))


def build_bass(kp):
    """kp: (15, 3) float32 numpy kernel points (runtime values baked)."""
    nc = bass.Bass(dynamic_dma_scratch_size=32768, num_swdge_queues=4)

    table = nc.dram_tensor("table", [M, ROW16], I16, kind="ExternalInput")
    rel_in = nc.dram_tensor("rel3", [128, N_KG * 3 * GG], F32,
                            kind="ExternalInput")
    idx_in = nc.dram_tensor("idx", [128, NK_CORE // 16], I16,
                            kind="ExternalInput")
    wp_in = nc.dram_tensor("wp16", [C_IN, P * C_OUT], F16,
                           kind="ExternalInput")
    bias_in = nc.dram_tensor("bias16", [1, C_OUT], F16, kind="ExternalInput")
    mask120_in = nc.dram_tensor("mask120", [128, 120], F16,
                                kind="ExternalInput")
    ident_in = nc.dram_tensor("ident16", [128, 128], F32,
                              kind="ExternalInput")
    kpb_in = nc.dram_tensor("kpb", [128, 48], F32, kind="ExternalInput")
    cnt_in = nc.dram_tensor("cnt16", [1, NQ_CORE], F16, kind="ExternalInput")
    cinv_in = nc.dram_tensor("cinv", [128, NQ_CORE // 128], F32,
                             kind="ExternalInput")
    out_t = nc.dram_tensor("out", [NQ_CORE, C_OUT], F32,
                           kind="ExternalOutput")

    nc.gpsimd.load_library(library_config.mlp)

    with TileContext(nc) as tc:
        with tc.tile_pool(name="const", bufs=1) as cpool, \
             tc.tile_pool(name="gath", bufs=4) as gpool, \
             tc.tile_pool(name="idxp", bufs=2) as ipool, \
             tc.tile_pool(name="kwp", bufs=2) as kwpool, \
             tc.tile_pool(name="kbd", bufs=1) as kbpool, \
             tc.tile_pool(name="wt", bufs=2) as wtpool, \
             tc.tile_pool(name="sm", bufs=3) as smpool, \
             tc.tile_pool(name="df", bufs=2) as dfpool, \
             tc.tile_pool(name="fin", bufs=3) as fpool, \
             tc.tile_pool(name="ps1", bufs=4, space="PSUM") as ps1pool, \
             tc.tile_pool(name="ps2", bufs=4, space="PSUM") as ps2pool, \
             tc.tile_pool(name="ps3", bufs=2, space="PSUM") as ps3pool:

            # ---- constants ----
            wp_t = cpool.tile([C_IN, P * C_OUT], F16, tag="wp")
            nc.sync.dma_start(wp_t[:], wp_in[:])
            bias_t = cpool.tile([1, C_OUT], F16, tag="bias")
            nc.sync.dma_start(bias_t[:], bias_in[:])
            mask120_t = cpool.tile([128, 120], F16, tag="m120")
            nc.sync.dma_start(mask120_t[:], mask120_in[:])
            kpb_t = cpool.tile([128, 48], F32, tag="kpb")
            nc.sync.dma_start(kpb_t[:], kpb_in[:])
            cinv_t = cpool.tile([128, NQ_CORE // 128], F32, tag="cinv")
            nc.sync.dma_start(cinv_t[:], cinv_in[:])
            nidx_reg = nc.gpsimd.to_reg(1024)
            pending_e2 = None

            prev = None
            for kg in range(N_KG):
                # ---- gathers: 1 idx DMA + 16 chunks of 1024 idx ----
                idxg = ipool.tile([128, GG * 8], I16, tag="idxg")
                nc.sync.dma_start(
                    idxg[:], idx_in[:, kg * GG * 8:(kg + 1) * GG * 8])
                gts = []
                for h in range(2):
                    gth = gpool.tile([128, G_ST, ROW16], I16, tag="gath")
                    for g in range(8):
                        gg = h * 8 + g
                        nc.gpsimd.dma_gather(
                            gth[:, g * 8:(g + 1) * 8, :], table[:],
                            idxg[:, gg * 64:(gg + 1) * 64],
                            1024, nidx_reg, ROW16, queue_num=gg % 4)
                    gts.append(gth[:].bitcast(F16))

                # ---- rel3 = s - q (host-precomputed), SoA [128, 3, GG] ----
                rel3 = smpool.tile([128, GG, 3], F32, tag="rel3")
                nc.sync.dma_start(
                    rel3[:].rearrange("p g d -> p (g d)"),
                    rel_in[:, kg * 3 * GG:(kg + 1) * 3 * GG])
                cntg = smpool.tile([1, 2, 512], F16, tag="cntg")
                nc.sync.dma_start(
                    cntg[:].rearrange("p a b -> p (a b)"),
                    cnt_in[:, kg * 1024:(kg + 1) * 1024])

                # ---- d2[g,p] = sum_d (rel_d - kp[p,d])^2, bulk ops ----
                kwt = kwpool.tile([128, GG, P], F32, tag="kw")
                QQ = GG // 4
                for h in range(4):
                    diff = dfpool.tile([128, QQ * 45], F32, tag="diff")
                    nc.vector.tensor_tensor(
                        out=ap_view(diff[:], 0, [[45, QQ], [3, P], [1, 3]]),
                        in0=ap_view(rel3[:], h * QQ * 3,
                                    [[3, QQ], [0, P], [1, 3]]),
                        in1=ap_view(kpb_t[:], 0, [[0, QQ], [3, P], [1, 3]]),
                        op=mybir.AluOpType.add)
                    nc.scalar.activation(diff[:], diff[:],
                                         mybir.ActivationFunctionType.Square,
                                         bias=0.0, scale=1.0)
                    nc.vector.tensor_reduce(
                        out=ap_view(kwt[:], h * QQ * P, [[1, QQ * P]]),
                        in_=ap_view(diff[:], 0, [[3, QQ * P], [1, 3]]),
                        axis=mybir.AxisListType.X, op=mybir.AluOpType.add)

                # kw = relu(1 - sqrt(d2 + 1e-10)/sigma), fp16 out
                nc.scalar.activation(kwt[:], kwt[:],
                                     mybir.ActivationFunctionType.Sqrt,
                                     bias=kpb_t[:, 45:46], scale=1.0)
                kwt16 = kwpool.tile([128, GG, P], F16, tag="kw16")
                nc.scalar.activation(kwt16[:], kwt[:],
                                     mybir.ActivationFunctionType.Relu,
                                     bias=1.0, scale=kpb_t[:, 46:47])
                # build all 4 kwbd halves now so DVE runs them before the
                # previous group's evictions (otherwise e1 stalls ~9us/kg)
                kbds = []
                for sti in range(KW_ST):
                    pair = []
                    for hf in range(2):
                        kb = kbpool.tile([128, 3840], F16,
                                         tag=f"kbd{sti}{hf}")
                        bl0 = sti * G_ST + hf * 32
                        nc.vector.tensor_tensor(
                            out=ap_view(kb[:], 0,
                                        [[120, 32], [15, 8], [1, 15]]),
                            in0=ap_view(kwt16[:], bl0 * P,
                                        [[P, 32], [0, 8], [1, P]]),
                            in1=ap_view(mask120_t[:], 0,
                                        [[0, 32], [15, 8], [1, 15]]),
                            op=mybir.AluOpType.mult)
                        pair.append(kb)
                    kbds.append(pair)

                cur = (kg, gts, kbds, cntg)
                if prev is None:
                    prev = cur
                    continue
                kg_c, gts, kbds, cntg = prev
                prev = cur

                for sti in range(KW_ST):
                    st = kg_c * KW_ST + sti
                    kbd, kbd2 = kbds[sti]
                    # einsum1: 16 bank groups of 4 blocks (32 queries)
                    wtt = wtpool.tile([C_IN, 7680], F16, tag="wt")
                    for bg in range(16):
                        pse1 = ps1pool.tile([C_IN, 480], F32, tag="pse1")
                        for j in range(4):
                            bl = bg * 4 + j
                            kb = kbd if bl < 32 else kbd2
                            kbl = bl % 32
                            nc.tensor.matmul(
                                pse1[:, j * 120:(j + 1) * 120],
                                ap_view(gts[sti], bl * ROW16, [[1, C_IN]]),
                                ap_view(kb[:], kbl * 120, [[1, 120]]),
                                start=True, stop=True)
                        nc.vector.tensor_copy(
                            wtt[:, bg * 480:bg * 480 + 240],
                            pse1[:, 0:240])
                        nc.scalar.copy(
                            wtt[:, bg * 480 + 240:bg * 480 + 480],
                            pse1[:, 240:480])

                    # einsum2 deferred one ST (keeps PE fed)
                    if pending_e2 is not None:
                        _emit_e2(nc, pending_e2, ps2pool, fpool,
                                 wp_t, bias_t, cinv_t, out_t)
                    pending_e2 = (st, wtt, cntg, sti)

            # drain: ST stage of the last kg
            kg_c, gts, kbds, cntg = prev
            for sti in range(KW_ST):
                st = kg_c * KW_ST + sti
                kbd, kbd2 = kbds[sti]
                wtt = wtpool.tile([C_IN, 7680], F16, tag="wt")
                for bg in range(16):
                    pse1 = ps1pool.tile([C_IN, 480], F32, tag="pse1")
                    for j in range(4):
                        bl = bg * 4 + j
                        kb = kbd if bl < 32 else kbd2
                        kbl = bl % 32
                        nc.tensor.matmul(
                            pse1[:, j * 120:(j + 1) * 120],
                            ap_view(gts[sti], bl * ROW16, [[1, C_IN]]),
                            ap_view(kb[:], kbl * 120, [[1, 120]]),
                            start=True, stop=True)
                    nc.vector.tensor_copy(
                        wtt[:, bg * 480:bg * 480 + 240],
                        pse1[:, 0:240])
                    nc.scalar.copy(
                        wtt[:, bg * 480 + 240:bg * 480 + 480],
                        pse1[:, 240:480])
                if pending_e2 is not None:
                    _emit_e2(nc, pending_e2, ps2pool, fpool,
                             wp_t, bias_t, cinv_t, out_t)
                pending_e2 = (st, wtt, cntg, sti)

            if pending_e2 is not None:
                _emit_e2(nc, pending_e2, ps2pool, fpool,
                         wp_t, bias_t, cinv_t, out_t)
    return nc


def _emit_e2(nc, pending_e2, ps2pool, fpool, wp_t, bias_t,
             cinv_t, out_t):
    st, wtt, cntg, sti = pending_e2
    for qc in range(4):
        pse2 = ps2pool.tile([128, 128], F32, tag="pse2")
        for p in range(P):
            nc.tensor.matmul(
                pse2[:],
                ap_view(wtt[:], qc * 1920 + p, [[15, 128]]),
                ap_view(wp_t[:], p * C_OUT, [[1, C_OUT]]),
                start=(p == 0), stop=False)
        nc.tensor.matmul(
            pse2[:],
            ap_view(cntg[:], sti * 512 + qc * 128, [[1, 128]]),
            bias_t[:], start=False, stop=True)
        trsb = fpool.tile([128, 128], F32, tag="trsb")
        cc = st * 4 + qc
        nc.scalar.activation(
            trsb[:], pse2[:], mybir.ActivationFunctionType.Copy,
            bias=0.0, scale=cinv_t[:, cc:cc + 1])
        n0 = st * 512 + qc * 128
        nc.sync.dma_start(out_t[n0:n0 + 128, :], trsb[:])


def _make_runner(nc, n_cores):
    bass2jax.install_neuronx_cc_hook()
    import jax
    from jax.sharding import Mesh, PartitionSpec
    from jax.experimental.shard_map import shard_map

    partition_name = (nc.partition_id_tensor.name
                      if nc.partition_id_tensor else None)
    in_names, out_names, out_avals, zero_outs = [], [], [], []
    for alloc in nc.m.functions[0].allocations:
        if not isinstance(alloc, mybir.MemoryLocationSet):
            continue
        name = alloc.memorylocations[0].name
        if alloc.kind == "ExternalInput":
            if name != partition_name:
                in_names.append(name)
        elif alloc.kind == "ExternalOutput":
            shape = tuple(alloc.tensor_shape)
            dtype = mybir.dt.np(alloc.dtype)
            out_names.append(name)
            out_avals.append(jax.core.ShapedArray(shape, dtype))
            zero_outs.append(np.zeros(shape, dtype))
    n_params = len(in_names)
    n_outs = len(out_avals)
    all_in = in_names + out_names + ([partition_name] if partition_name else [])

    def _body(*args):
        operands = list(args)
        if partition_name is not None:
            operands.append(bass2jax.partition_id_tensor())
        outs = bass2jax._bass_exec_p.bind(
            *operands, out_avals=tuple(out_avals), in_names=tuple(all_in),
            out_names=tuple(out_names), lowering_input_output_aliases=(),
            sim_require_finite=False, sim_require_nnan=False, nc=nc)
        return tuple(outs)

    devices = jax.devices()[:n_cores]
    mesh = Mesh(np.asarray(devices), ("core",))
    in_specs = (PartitionSpec("core"),) * (n_params + n_outs)
    out_specs = (PartitionSpec("core"),) * n_outs
    jit_fn = jax.jit(
        shard_map(_body, mesh=mesh, in_specs=in_specs, out_specs=out_specs,
                  check_rep=False), keep_unused=True)

    def run(in_maps):
        per_core = [[np.asarray(m[n]) for n in in_names] for m in in_maps]
        args = [np.concatenate([per_core[c][i] for c in range(n_cores)],
                               axis=0)
                for i in range(n_params)]
        args += [np.zeros((n_cores * z.shape[0], *z.shape[1:]), z.dtype)
                 for z in zero_outs]
        outs = [np.asarray(o) for o in jit_fn(*args)]
        return [{n: outs[i].reshape(n_cores, *out_avals[i].shape)[c]
                 for i, n in enumerate(out_names)}
                for c in range(n_cores)], jit_fn, args

    return run


_BUILT = {}


def _get_runner(kp):
    key = kp.tobytes()
    if key not in _BUILT:
        nc = build_bass(kp)
        _BUILT[key] = _make_runner(nc, N_CORES)
    return _BUILT[key]


def _host_prep(query_points, support_points, support_features,
               neighbor_indices, weights, bias, kernel_points):
    qp = np.asarray(query_points, np.float32)
    sp = np.asarray(support_points, np.float32)
    sf = np.asarray(support_features, np.float32)
    ni = np.asarray(neighbor_indices)
    ni = np.clip(ni, 0, M - 1)
    w = np.ascontiguousarray(np.asarray(weights, np.float32))
    # wp16: [c, (p, o)] fp16
    wp16 = np.ascontiguousarray(
        w.transpose(1, 0, 2).reshape(C_IN, P * C_OUT)).astype(np.float16)
    bias16 = np.asarray(bias, np.float32).reshape(1, C_OUT).astype(np.float16)

    mask120 = np.zeros((128, 120), np.float16)
    for q in range(8):
        mask120[q * 16:(q + 1) * 16, q * 15:(q + 1) * 15] = 1.0
    ident16 = np.eye(128, dtype=np.float32)
    kpv = np.asarray(kernel_points, np.float32)
    kpb = np.zeros((128, 48), np.float32)
    for p in range(P):
        for d in range(3):
            kpb[:, 3 * p + d] = -kpv[p, d]
    kpb[:, 45] = 1e-10
    kpb[:, 46] = -1.0 / SIGMA

    # tables (per batch): [M, 128] f16 = feats f16 | coords f32 at 32..34
    tables = []
    has_feat = np.abs(sf).sum(axis=2) > 0          # [B, M]
    for b in range(B):
        t16 = np.zeros((M, ROW16), np.float16)
        t16[:, :C_IN] = sf[b].astype(np.float16)
        v32 = t16.view(np.float32)
        v32[:, 32:35] = sp[b]
        tables.append(t16.view(np.int16))

    in_maps = []
    for c in range(N_CORES):
        b, half = divmod(c, 2)
        n0 = half * NQ_CORE
        idx = ni[b, n0:n0 + NQ_CORE, :].reshape(NK_CORE)
        idx_l = idx.reshape(NK_CORE // 16, 16).T.astype(np.int16)
        idx_l = np.tile(idx_l, (8, 1))                    # [128, NK/16]
        qrep = np.repeat(qp[b, n0:n0 + NQ_CORE, :], K, axis=0)
        relf = sp[b][idx] - qrep                          # [NK, 3]
        rel3 = relf.reshape(N_KG, GG, 128, 3).transpose(2, 0, 1, 3)
        rel3 = np.ascontiguousarray(rel3.reshape(128, N_KG * 3 * GG))
        # exact neighbor counts
        cnt = np.maximum(
            has_feat[b][ni[b, n0:n0 + NQ_CORE, :]].sum(axis=1), 1.0)
        cnt16 = cnt.astype(np.float16).reshape(1, NQ_CORE)
        cinv = (1.0 / cnt).astype(np.float32).reshape(
            NQ_CORE // 128, 128).T  # [128, 64]
        in_maps.append({
            "table": tables[b], "rel3": rel3,
            "idx": np.ascontiguousarray(idx_l),
            "wp16": wp16, "bias16": bias16, "mask120": mask120,
            "ident16": ident16, "kpb": kpb, "cnt16": cnt16,
            "cinv": np.ascontiguousarray(cinv),
        })
    return in_maps


def kernel(query_points, support_points, support_features, neighbor_indices,
           weights, bias, kernel_points):
    kp = np.asarray(kernel_points, np.float32)
    run = _get_runner(kp)
    in_maps = _host_prep(query_points, support_points, support_features,
                         neighbor_indices, weights, bias, kernel_points)
    results, _, _ = run(in_maps)
    out = np.zeros((B, N, C_OUT), np.float32)
    for c in range(N_CORES):
        b, half = divmod(c, 2)
        n0 = half * NQ_CORE
        out[b, n0:n0 + NQ_CORE, :] = results[c]["out"]
    return out


# revision 30
# speedup vs baseline: 1.0949x; 1.0949x over previous
"""KPConv (nn_KPConvFPN) Trainium2 Bass kernel — v2.

Sharding: 8 cores; core c handles batch b=c//2, query half (c%2)*8192.
Host prep packs a combined gather table in DRAM (input): row m (256B) =
[64 x fp16 feat | sx,sy,sz as fp32 at f32-cols 32..34 | pad], plus exact
per-query neighbor counts (computed host-side from inputs).

Per-core pipeline, per kw-group kg (1024 queries, 16 gather chunks):
  1. one idx DMA [128, 1024] i16; 16 x dma_gather (SWDGE) of 1024 rows
     each, round-robin over 4 queues (queues run on distinct Q7 core
     pairs and overlap ~3x).
  2. rel3 [128, 3, GG] = s - q (SoA so ACT reads are contiguous).
  3. kw[n,k,p] = relu(1 - sqrt(|rel - kp_p|^2)/sigma) via ACT/DVE,
     fp16 output.
  4. kbd = kw * blockdiag-mask (fp16), einsum1 on PE per 8-query block:
     lhsT = gathered feats fp16, rhs = kbd -> PSUM, evict fp16 to wtt.
  5. einsum2 on PE fp16: 15 x (W_p stationary) + bias x cnt row,
     accumulate [o=128, q=512] in PSUM.
  6. PE-transpose 128-col chunks; ACT copy with per-partition scale
     1/cnt; store.
"""
import json

import numpy as np

import concourse.bass as bass
import concourse.mybir as mybir
from concourse.tile import TileContext
from concourse import library_config
from concourse import bass2jax

F32 = mybir.dt.float32
F16 = mybir.dt.float16
I16 = mybir.dt.int16

B, N, M, K = 4, 16384, 16384, 16
C_IN, C_OUT, P = 64, 128, 15
SIGMA = 0.03
N_CORES = 8
NQ_CORE = N // 2            # 8192 queries per core
NK_CORE = NQ_CORE * K       # 131072 gathered rows per core
ST_Q = 512                  # queries per supertile
N_ST = NQ_CORE // ST_Q      # 16
KW_ST = 2                   # supertiles per kw group
N_KG = N_ST // KW_ST        # 8
GG = KW_ST * ST_Q * K // 128  # 128 g-cols per kw group
G_ST = GG // KW_ST          # 64 g-cols per supertile
ROW16 = 128                 # fp16 units per table row (256B)

# ---------------------------------------------------------------------------
# walrus workaround: this nix walrus build supports ONE sync-wait per
# instruction; split extra waits onto NoOps inserted before the offender
# (same-engine program order preserves semantics). Also run
# codegen_inst_isa_subclasses (Bacc does; raw Bass doesn't) so extended
# instructions get their ISA bytes.
_orig_to_json_bytes = bass.Bass.to_json_bytes


def _fix_block(bb, ctr):
    insts = bb.get("instructions")
    if not isinstance(insts, list):
        return
    new = []
    for inst in insts:
        si = inst.get("sync_info")
        ow = si.get("on_wait") if isinstance(si, dict) else None
        if ow and len(ow) > 1:
            for w in ow[:-1]:
                ctr[0] += 1
                nop = {"engine": inst["engine"], "ins": [], "outs": [],
                       "name": f"I-wsplit-{ctr[0]}", "opcode": "NoOp",
                       "sync_info": {"on_update": [], "on_wait": [w]},
                       "text_hint": "wsplit"}
                if "debug" in inst:
                    nop["debug"] = inst["debug"]
                new.append(nop)
            si["on_wait"] = [ow[-1]]
        new.append(inst)
    bb["instructions"] = new


def _walk(o, ctr):
    if isinstance(o, dict):
        if isinstance(o.get("instructions"), list):
            _fix_block(o, ctr)
        for v in o.values():
            _walk(v, ctr)
    elif isinstance(o, list):
        for v in o:
            _walk(v, ctr)


def _to_json_bytes_split(self):
    mybir.codegen_inst_isa_subclasses(self)
    raw = _orig_to_json_bytes(self)
    d = json.loads(raw)
    ctr = [0]
    _walk(d, ctr)
    return json.dumps(d).encode()


bass.Bass.to_json_bytes = _to_json_bytes_split


def ap_view(t_ap, extra_offset, dims):
    """AP over tile t_ap with explicit free dims [[step, count], ...]
    (steps in elements); partition dim is taken from the tile."""
    return bass.AP(t_ap.tensor, t_ap.offset + extra_offset,
                   [t_ap.ap[0]] + list(dims))


def build_bass(kp):
    """kp: (15, 3) float32 numpy kernel points (runtime values baked)."""
    nc = bass.Bass(dynamic_dma_scratch_size=32768, num_swdge_queues=4)

    table = nc.dram_tensor("table", [M, ROW16], I16, kind="ExternalInput")
    rel_in = nc.dram_tensor("rel3", [128, N_KG * 3 * GG], F32,
                            kind="ExternalInput")
    idx_in = nc.dram_tensor("idx", [128, NK_CORE // 16], I16,
                            kind="ExternalInput")
    wp_in = nc.dram_tensor("wp16", [C_IN, P * C_OUT], F16,
                           kind="ExternalInput")
    bias_in = nc.dram_tensor("bias16", [1, C_OUT], F16, kind="ExternalInput")
    mask120_in = nc.dram_tensor("mask120", [128, 120], F16,
                                kind="ExternalInput")
    ident_in = nc.dram_tensor("ident16", [128, 128], F32,
                              kind="ExternalInput")
    kpb_in = nc.dram_tensor("kpb", [128, 48], F32, kind="ExternalInput")
    cnt_in = nc.dram_tensor("cnt16", [1, NQ_CORE], F16, kind="ExternalInput")
    cinv_in = nc.dram_tensor("cinv", [128, NQ_CORE // 128], F32,
                             kind="ExternalInput")
    out_t = nc.dram_tensor("out", [NQ_CORE, C_OUT], F32,
                           kind="ExternalOutput")

    nc.gpsimd.load_library(library_config.mlp)

    with TileContext(nc) as tc:
        with tc.tile_pool(name="const", bufs=1) as cpool, \
             tc.tile_pool(name="gath", bufs=4) as gpool, \
             tc.tile_pool(name="idxp", bufs=2) as ipool, \
             tc.tile_pool(name="kwp", bufs=2) as kwpool, \
             tc.tile_pool(name="kbd", bufs=2) as kbpool, \
             tc.tile_pool(name="wt", bufs=2) as wtpool, \
             tc.tile_pool(name="sm", bufs=3) as smpool, \
             tc.tile_pool(name="df", bufs=2) as dfpool, \
             tc.tile_pool(name="fin", bufs=3) as fpool, \
             tc.tile_pool(name="ps1", bufs=4, space="PSUM") as ps1pool, \
             tc.tile_pool(name="ps2", bufs=4, space="PSUM") as ps2pool, \
             tc.tile_pool(name="ps3", bufs=2, space="PSUM") as ps3pool:

            # ---- constants ----
            wp_t = cpool.tile([C_IN, P * C_OUT], F16, tag="wp")
            nc.sync.dma_start(wp_t[:], wp_in[:])
            bias_t = cpool.tile([1, C_OUT], F16, tag="bias")
            nc.sync.dma_start(bias_t[:], bias_in[:])
            mask120_t = cpool.tile([128, 120], F16, tag="m120")
            nc.sync.dma_start(mask120_t[:], mask120_in[:])
            kpb_t = cpool.tile([128, 48], F32, tag="kpb")
            nc.sync.dma_start(kpb_t[:], kpb_in[:])
            cinv_t = cpool.tile([128, NQ_CORE // 128], F32, tag="cinv")
            nc.sync.dma_start(cinv_t[:], cinv_in[:])
            nidx_reg = nc.gpsimd.to_reg(1024)
            pending_e2 = None

            prev = None
            for kg in range(N_KG):
                # ---- gathers: 1 idx DMA + 16 chunks of 1024 idx ----
                idxg = ipool.tile([128, GG * 8], I16, tag="idxg")
                nc.sync.dma_start(
                    idxg[:], idx_in[:, kg * GG * 8:(kg + 1) * GG * 8])
                gts = []
                for h in range(2):
                    gth = gpool.tile([128, G_ST, ROW16], I16, tag="gath")
                    for g in range(8):
                        gg = h * 8 + g
                        nc.gpsimd.dma_gather(
                            gth[:, g * 8:(g + 1) * 8, :], table[:],
                            idxg[:, gg * 64:(gg + 1) * 64],
                            1024, nidx_reg, ROW16, queue_num=gg % 4)
                    gts.append(gth[:].bitcast(F16))

                # ---- rel3 = s - q (host-precomputed), SoA [128, 3, GG] ----
                rel3 = smpool.tile([128, GG, 3], F32, tag="rel3")
                nc.sync.dma_start(
                    rel3[:].rearrange("p g d -> p (g d)"),
                    rel_in[:, kg * 3 * GG:(kg + 1) * 3 * GG])
                cntg = smpool.tile([1, 2, 512], F16, tag="cntg")
                nc.sync.dma_start(
                    cntg[:].rearrange("p a b -> p (a b)"),
                    cnt_in[:, kg * 1024:(kg + 1) * 1024])

                # ---- d2[g,p] = sum_d (rel_d - kp[p,d])^2, bulk ops ----
                kwt = kwpool.tile([128, GG, P], F32, tag="kw")
                QQ = GG // 4
                for h in range(4):
                    diff = dfpool.tile([128, QQ * 45], F32, tag="diff")
                    nc.vector.tensor_tensor(
                        out=ap_view(diff[:], 0, [[45, QQ], [3, P], [1, 3]]),
                        in0=ap_view(rel3[:], h * QQ * 3,
                                    [[3, QQ], [0, P], [1, 3]]),
                        in1=ap_view(kpb_t[:], 0, [[0, QQ], [3, P], [1, 3]]),
                        op=mybir.AluOpType.add)
                    nc.scalar.activation(diff[:], diff[:],
                                         mybir.ActivationFunctionType.Square,
                                         bias=0.0, scale=1.0)
                    nc.vector.tensor_reduce(
                        out=ap_view(kwt[:], h * QQ * P, [[1, QQ * P]]),
                        in_=ap_view(diff[:], 0, [[3, QQ * P], [1, 3]]),
                        axis=mybir.AxisListType.X, op=mybir.AluOpType.add)

                # kw = relu(1 - sqrt(d2 + 1e-10)/sigma), fp16 out
                nc.scalar.activation(kwt[:], kwt[:],
                                     mybir.ActivationFunctionType.Sqrt,
                                     bias=kpb_t[:, 45:46], scale=1.0)
                kwt16 = kwpool.tile([128, GG, P], F16, tag="kw16")
                nc.scalar.activation(kwt16[:], kwt[:],
                                     mybir.ActivationFunctionType.Relu,
                                     bias=1.0, scale=kpb_t[:, 46:47])
                cur = (kg, gts, kwt16, cntg)
                if prev is None:
                    prev = cur
                    continue
                kg_c, gts, kwt16, cntg = prev
                prev = cur

                for sti in range(KW_ST):
                    st = kg_c * KW_ST + sti
                    kbd = kbpool.tile([128, 3840], F16, tag="kbd")
                    kbd2 = kbpool.tile([128, 3840], F16, tag="kbd2")
                    for hf, kb in ((0, kbd), (1, kbd2)):
                        bl0 = sti * G_ST + hf * 32
                        nc.vector.tensor_tensor(
                            out=ap_view(kb[:], 0,
                                        [[120, 32], [15, 8], [1, 15]]),
                            in0=ap_view(kwt16[:], bl0 * P,
                                        [[P, 32], [0, 8], [1, P]]),
                            in1=ap_view(mask120_t[:], 0,
                                        [[0, 32], [15, 8], [1, 15]]),
                            op=mybir.AluOpType.mult)
                    # einsum1: 16 bank groups of 4 blocks (32 queries)
                    wtt = wtpool.tile([C_IN, 7680], F16, tag="wt")
                    for bg in range(16):
                        pse1 = ps1pool.tile([C_IN, 480], F32, tag="pse1")
                        for j in range(4):
                            bl = bg * 4 + j
                            kb = kbd if bl < 32 else kbd2
                            kbl = bl % 32
                            nc.tensor.matmul(
                                pse1[:, j * 120:(j + 1) * 120],
                                ap_view(gts[sti], bl * ROW16, [[1, C_IN]]),
                                ap_view(kb[:], kbl * 120, [[1, 120]]),
                                start=True, stop=True)
                        nc.vector.tensor_copy(
                            wtt[:, bg * 480:bg * 480 + 240],
                            pse1[:, 0:240])
                        nc.scalar.copy(
                            wtt[:, bg * 480 + 240:bg * 480 + 480],
                            pse1[:, 240:480])

                    # einsum2 deferred one ST (keeps PE fed)
                    if pending_e2 is not None:
                        _emit_e2(nc, pending_e2, ps2pool, fpool,
                                 wp_t, bias_t, cinv_t, out_t)
                    pending_e2 = (st, wtt, cntg, sti)

            # drain: ST stage of the last kg
            kg_c, gts, kwt16, cntg = prev
            for sti in range(KW_ST):
                st = kg_c * KW_ST + sti
                kbd = kbpool.tile([128, 3840], F16, tag="kbd")
                kbd2 = kbpool.tile([128, 3840], F16, tag="kbd2")
                for hf, kb in ((0, kbd), (1, kbd2)):
                    bl0 = sti * G_ST + hf * 32
                    nc.vector.tensor_tensor(
                        out=ap_view(kb[:], 0,
                                    [[120, 32], [15, 8], [1, 15]]),
                        in0=ap_view(kwt16[:], bl0 * P,
                                    [[P, 32], [0, 8], [1, P]]),
                        in1=ap_view(mask120_t[:], 0,
                                    [[0, 32], [15, 8], [1, 15]]),
                        op=mybir.AluOpType.mult)
                wtt = wtpool.tile([C_IN, 7680], F16, tag="wt")
                for bg in range(16):
                    pse1 = ps1pool.tile([C_IN, 480], F32, tag="pse1")
                    for j in range(4):
                        bl = bg * 4 + j
                        kb = kbd if bl < 32 else kbd2
                        kbl = bl % 32
                        nc.tensor.matmul(
                            pse1[:, j * 120:(j + 1) * 120],
                            ap_view(gts[sti], bl * ROW16, [[1, C_IN]]),
                            ap_view(kb[:], kbl * 120, [[1, 120]]),
                            start=True, stop=True)
                    nc.vector.tensor_copy(
                        wtt[:, bg * 480:bg * 480 + 240],
                        pse1[:, 0:240])
                    nc.scalar.copy(
                        wtt[:, bg * 480 + 240:bg * 480 + 480],
                        pse1[:, 240:480])
                if pending_e2 is not None:
                    _emit_e2(nc, pending_e2, ps2pool, fpool,
                             wp_t, bias_t, cinv_t, out_t)
                pending_e2 = (st, wtt, cntg, sti)

            if pending_e2 is not None:
                _emit_e2(nc, pending_e2, ps2pool, fpool,
                         wp_t, bias_t, cinv_t, out_t)
    return nc


def _emit_e2(nc, pending_e2, ps2pool, fpool, wp_t, bias_t,
             cinv_t, out_t):
    st, wtt, cntg, sti = pending_e2
    for qc in range(4):
        pse2 = ps2pool.tile([128, 128], F32, tag="pse2")
        for p in range(P):
            nc.tensor.matmul(
                pse2[:],
                ap_view(wtt[:], qc * 1920 + p, [[15, 128]]),
                ap_view(wp_t[:], p * C_OUT, [[1, C_OUT]]),
                start=(p == 0), stop=False)
        nc.tensor.matmul(
            pse2[:],
            ap_view(cntg[:], sti * 512 + qc * 128, [[1, 128]]),
            bias_t[:], start=False, stop=True)
        trsb = fpool.tile([128, 128], F32, tag="trsb")
        cc = st * 4 + qc
        nc.scalar.activation(
            trsb[:], pse2[:], mybir.ActivationFunctionType.Copy,
            bias=0.0, scale=cinv_t[:, cc:cc + 1])
        n0 = st * 512 + qc * 128
        nc.sync.dma_start(out_t[n0:n0 + 128, :], trsb[:])


def _make_runner(nc, n_cores):
    bass2jax.install_neuronx_cc_hook()
    import jax
    from jax.sharding import Mesh, PartitionSpec
    from jax.experimental.shard_map import shard_map

    partition_name = (nc.partition_id_tensor.name
                      if nc.partition_id_tensor else None)
    in_names, out_names, out_avals, zero_outs = [], [], [], []
    for alloc in nc.m.functions[0].allocations:
        if not isinstance(alloc, mybir.MemoryLocationSet):
            continue
        name = alloc.memorylocations[0].name
        if alloc.kind == "ExternalInput":
            if name != partition_name:
                in_names.append(name)
        elif alloc.kind == "ExternalOutput":
            shape = tuple(alloc.tensor_shape)
            dtype = mybir.dt.np(alloc.dtype)
            out_names.append(name)
            out_avals.append(jax.core.ShapedArray(shape, dtype))
            zero_outs.append(np.zeros(shape, dtype))
    n_params = len(in_names)
    n_outs = len(out_avals)
    all_in = in_names + out_names + ([partition_name] if partition_name else [])

    def _body(*args):
        operands = list(args)
        if partition_name is not None:
            operands.append(bass2jax.partition_id_tensor())
        outs = bass2jax._bass_exec_p.bind(
            *operands, out_avals=tuple(out_avals), in_names=tuple(all_in),
            out_names=tuple(out_names), lowering_input_output_aliases=(),
            sim_require_finite=False, sim_require_nnan=False, nc=nc)
        return tuple(outs)

    devices = jax.devices()[:n_cores]
    mesh = Mesh(np.asarray(devices), ("core",))
    in_specs = (PartitionSpec("core"),) * (n_params + n_outs)
    out_specs = (PartitionSpec("core"),) * n_outs
    jit_fn = jax.jit(
        shard_map(_body, mesh=mesh, in_specs=in_specs, out_specs=out_specs,
                  check_rep=False), keep_unused=True)

    def run(in_maps):
        per_core = [[np.asarray(m[n]) for n in in_names] for m in in_maps]
        args = [np.concatenate([per_core[c][i] for c in range(n_cores)],
                               axis=0)
                for i in range(n_params)]
        args += [np.zeros((n_cores * z.shape[0], *z.shape[1:]), z.dtype)
                 for z in zero_outs]
        outs = [np.asarray(o) for o in jit_fn(*args)]
        return [{n: outs[i].reshape(n_cores, *out_avals[i].shape)[c]
                 for i, n in enumerate(out_names)}
                for c in range(n_cores)], jit_fn, args

    return run


_BUILT = {}


def _get_runner(kp):
    key = kp.tobytes()
    if key not in _BUILT:
        nc = build_bass(kp)
        _BUILT[key] = _make_runner(nc, N_CORES)
    return _BUILT[key]


def _host_prep(query_points, support_points, support_features,
               neighbor_indices, weights, bias, kernel_points):
    qp = np.asarray(query_points, np.float32)
    sp = np.asarray(support_points, np.float32)
    sf = np.asarray(support_features, np.float32)
    ni = np.asarray(neighbor_indices)
    ni = np.clip(ni, 0, M - 1)
    w = np.ascontiguousarray(np.asarray(weights, np.float32))
    # wp16: [c, (p, o)] fp16
    wp16 = np.ascontiguousarray(
        w.transpose(1, 0, 2).reshape(C_IN, P * C_OUT)).astype(np.float16)
    bias16 = np.asarray(bias, np.float32).reshape(1, C_OUT).astype(np.float16)

    mask120 = np.zeros((128, 120), np.float16)
    for q in range(8):
        mask120[q * 16:(q + 1) * 16, q * 15:(q + 1) * 15] = 1.0
    ident16 = np.eye(128, dtype=np.float32)
    kpv = np.asarray(kernel_points, np.float32)
    kpb = np.zeros((128, 48), np.float32)
    for p in range(P):
        for d in range(3):
            kpb[:, 3 * p + d] = -kpv[p, d]
    kpb[:, 45] = 1e-10
    kpb[:, 46] = -1.0 / SIGMA

    # tables (per batch): [M, 128] f16 = feats f16 | coords f32 at 32..34
    tables = []
    has_feat = np.abs(sf).sum(axis=2) > 0          # [B, M]
    for b in range(B):
        t16 = np.zeros((M, ROW16), np.float16)
        t16[:, :C_IN] = sf[b].astype(np.float16)
        v32 = t16.view(np.float32)
        v32[:, 32:35] = sp[b]
        tables.append(t16.view(np.int16))

    in_maps = []
    for c in range(N_CORES):
        b, half = divmod(c, 2)
        n0 = half * NQ_CORE
        idx = ni[b, n0:n0 + NQ_CORE, :].reshape(NK_CORE)
        idx_l = idx.reshape(NK_CORE // 16, 16).T.astype(np.int16)
        idx_l = np.tile(idx_l, (8, 1))                    # [128, NK/16]
        qrep = np.repeat(qp[b, n0:n0 + NQ_CORE, :], K, axis=0)
        relf = sp[b][idx] - qrep                          # [NK, 3]
        rel3 = relf.reshape(N_KG, GG, 128, 3).transpose(2, 0, 1, 3)
        rel3 = np.ascontiguousarray(rel3.reshape(128, N_KG * 3 * GG))
        # exact neighbor counts
        cnt = np.maximum(
            has_feat[b][ni[b, n0:n0 + NQ_CORE, :]].sum(axis=1), 1.0)
        cnt16 = cnt.astype(np.float16).reshape(1, NQ_CORE)
        cinv = (1.0 / cnt).astype(np.float32).reshape(
            NQ_CORE // 128, 128).T  # [128, 64]
        in_maps.append({
            "table": tables[b], "rel3": rel3,
            "idx": np.ascontiguousarray(idx_l),
            "wp16": wp16, "bias16": bias16, "mask120": mask120,
            "ident16": ident16, "kpb": kpb, "cnt16": cnt16,
            "cinv": np.ascontiguousarray(cinv),
        })
    return in_maps


def kernel(query_points, support_points, support_features, neighbor_indices,
           weights, bias, kernel_points):
    kp = np.asarray(kernel_points, np.float32)
    run = _get_runner(kp)
    in_maps = _host_prep(query_points, support_points, support_features,
                         neighbor_indices, weights, bias, kernel_points)
    results, _, _ = run(in_maps)
    out = np.zeros((B, N, C_OUT), np.float32)
    for c in range(N_CORES):
        b, half = divmod(c, 2)
        n0 = half * NQ_CORE
        out[b, n0:n0 + NQ_CORE, :] = results[c]["out"]
    return out
